# revision 23
# baseline (speedup 1.0000x reference)
"""AvgPool2d-as-Toeplitz kernel for Trainium2 (8 NeuronCores, SPMD).

Reference computes out = (enc_x * mask) @ W.T where W is the dense
Toeplitz matrix of conv2d with kernel ones(C,C,KH,KW)/(KH*KW) over the
flattened zero-padded input (C=16, KH=KW=2, stride 2, pad 1, H=W=32),
and mask zeroes the 1-pixel padding ring of each 34x34 channel image.

Structure exploited:
  W[(co,oi,oj), (ci,i,j)] = 0.25  iff  i in {2oi, 2oi+1} and j in {2oj, 2oj+1}
— independent of co, summed over every ci. With x viewed as
[B, C, 34, 34] and the mask folded in structurally (pooling windows
never read the masked border rows/columns):

  out[b, co, oi, oj] = 0.25 * sum_ci sum_window x[b, ci, i, j]
       over i in {2oi, 2oi+1} ∩ [1,32],  j in {2oj, 2oj+1} ∩ [1,32]

The result is numerically exact (fp32 throughout, ~1e-7 fro error).

Per-core plan (4 batches per core, batch-parallel across 8 cores), raw
bacc with manual semaphores, flat single-block emission (no Block
wrapper: its exit drain+barrier would only delay the NEFF epilogue),
optimized for latency:
  - Input DMA trimmed to image rows 1..32, one DMA per HWDGE queue
    (per-DMA pipe latency of ~1.5us dominates, so fewer/bigger
    transfers win over chunking): SP ring rows 1-17 (output half 1),
    ACT ring rows 18-32.
  - DVE does all vector work (GPSIMD tensor ops need a library swap
    and run ~2x slower; ACT-engine copies trigger an activation-table
    load on the ACT DMA ring): column-pair adds + border-column copies
    + row-pair adds per half, and both PSUM->SBUF copies.  GPSIMD only
    builds E[(b,ci),(b2,co)] = 0.25*(b==b2) with std-lib
    memset/affine_select, off the critical path.
  - PE: two fp32 matmuls (ci-sum + co-broadcast + *0.25), one per
    output half, into separate PSUM banks; half 1 (oi 0..8) launches
    while half 2's input is still in flight.
  - Output DMA split across both queues, issued as soon as each half
    is staged, with NO completion waits: the fixed multi-us NEFF
    teardown (semaphore-reset storm) overlaps the output DMA flight,
    and the runtime drains DGE queues before completion.
  - Engines execute with relaxed ordering, so readers that follow
    same-engine writers carry attached semaphore waits (dropping them
    corrupts the first elements written by the preceding op on cold
    runs).
"""

import sys

import numpy as np

if "/opt/trn_rl_repo" not in sys.path:
    sys.path.insert(0, "/opt/trn_rl_repo")

B, C = 32, 16
HP = WP = 34
OH = OW = 17
IMG = HP * WP             # 1156
IN_DIM = C * IMG          # 18496
OUT_DIM = C * OH * OW     # 4624
N_CORES = 8
B_SH = B // N_CORES       # 4 batches per core
P = B_SH * C              # 64 partitions in use

OI1 = 9                   # output rows in half 1 (oi 0..8 <- image rows 0..17)
N1 = OI1 * OW             # 153
N2 = (OH - OI1) * OW      # 136
GAP = 16                  # sacrificial staging gap between output halves
OFF2 = N1 + GAP           # staging offset of output half 2

RS = 18                   # image-row split: SP ring rows 1..17, ACT rows 18..32

_PROGRAM = None


def _build_program():
    import concourse.bacc as bacc
    import concourse.mybir as mybir

    f32 = mybir.dt.float32
    add = mybir.AluOpType.add
    nc = bacc.Bacc()

    x = nc.declare_dram_parameter("x", [B_SH, IN_DIM], f32, isOutput=False)
    out = nc.declare_dram_parameter("out", [B_SH, OUT_DIM], f32, isOutput=True)
    xv = x[:, :].rearrange("b (c f) -> (b c) f", c=C)   # [64, 1156]
    ov = out[:, :].rearrange("b (co s) -> (b co) s", co=C)

    with (
        nc.sbuf_tensor([P, IMG], f32) as xt,
        nc.sbuf_tensor([P, P], f32) as et,
        nc.sbuf_tensor([P, HP * OW], f32) as at,
        nc.sbuf_tensor([P, N1 + N2], f32) as a2t,
        nc.sbuf_tensor([P, OFF2 + N2], f32) as ot,
        nc.psum_tensor([P, N1], f32) as pt0,
        nc.psum_tensor([P, N2], f32) as pt1,
        nc.semaphore("s_a1") as s_a1,
        nc.semaphore("s_b1") as s_b1,
        nc.semaphore("s_gps") as s_gps,
        nc.semaphore("s_dve") as s_dve,
        nc.semaphore("s_pe") as s_pe,
        nc.semaphore("s_cp1") as s_cp1,
        nc.semaphore("s_cp2") as s_cp2,
        nc.semaphore("s_out") as s_out,
        nc.Block() as block,
    ):
        x3 = xt[:].rearrange("p (i j) -> p i j", i=HP)
        a3 = at[:].rearrange("p (i oj) -> p i oj", i=HP)
        a23a = a2t[:, 0:N1].rearrange("p (oi oj) -> p oi oj", oi=OI1)
        a23b = a2t[:, N1:N1 + N2].rearrange(
            "p (oi oj) -> p oi oj", oi=OH - OI1
        )
        e3 = et[:].rearrange("p (qb qc) -> p qb qc", qb=B_SH)

        def ctt(eng, r0, r1):
            # column-pair add for interior output columns oj 1..15
            return eng.tensor_tensor(
                a3[:, r0:r1, 1:16],
                x3[:, r0:r1, 2:32:2], x3[:, r0:r1, 3:33:2], add,
            )

        def cb(eng, r0, r1):
            # border output columns oj 0 / 16 <- image columns 1 / 32
            return eng.tensor_copy(
                a3[:, r0:r1, 0:17:16], x3[:, r0:r1, 1:33:31]
            )

        @block.sync
        def _(sync):
            # rows 1..17 (output half 1) on the SP ring
            sync.dma_start(
                xt[:, WP:RS * WP], xv[:, WP:RS * WP]
            ).then_inc(s_a1, 16)
            # ship output half 2 once staged; no completion wait: the NEFF
            # teardown overlaps the output flight and the runtime drains
            # the DGE queues before completion
            sync.wait_ge(s_cp2, 1)
            sync.dma_start(
                ov[:, N1:OH * OW], ot[:, OFF2:OFF2 + N2], single_packet=True
            ).then_inc(s_out, 16)

        @block.scalar
        def _(scalar):
            # rows 18..32 (output half 2) on the ACT ring
            scalar.dma_start(
                xt[:, RS * WP:IMG - WP], xv[:, RS * WP:IMG - WP]
            ).then_inc(s_b1, 16)
            scalar.wait_ge(s_cp1, 1)
            scalar.dma_start(
                ov[:, 0:N1], ot[:, 0:N1], single_packet=True
            ).then_inc(s_out, 16)

        @block.gpsimd
        def _(gpsimd):
            # masked image rows 0 and 33 of the column-pooled tile -> 0,
            # so the row-pair stage needs no border special-casing
            gpsimd.memset(a3[:, 0:HP:HP - 1, :], 0.0).then_inc(s_gps, 1)
            # E[p,(qb,qc)] = 0.25 iff 0 <= p - 16*qb <= 15
            gpsimd.memset(et[:], 0.25).then_inc(s_gps, 1)
            gpsimd.wait_ge(s_gps, 2)
            nc.gpsimd.affine_select(
                e3, e3, [[-C, B_SH], [0, C]], mybir.AluOpType.is_ge, 0.0,
                base=0, channel_multiplier=1,
            ).then_inc(s_gps, 1)
            gpsimd.wait_ge(s_gps, 3)
            nc.gpsimd.affine_select(
                e3, e3, [[C, B_SH], [0, C]], mybir.AluOpType.is_ge, 0.0,
                base=C - 1, channel_multiplier=-1,
            ).then_inc(s_gps, 1)

        @block.vector
        def _(vector):
            # round E to fp32r for the single-pass matmuls, and zero the
            # even-width pad column of a2 half 1 (fp32r needs even N)
            vector.wait_ge(s_gps, 4)
            nc.vector.tensor_copy(etr[:], et[:]).then_inc(s_dve, 1)
            nc.vector.tensor_copy(
                a2t[:, N1:N1 + 1], a3[:, 0, 0:1]
            ).then_inc(s_dve, 1)                          # s_dve = 2
            vector.wait_ge(s_a1, 16)
            ctt(nc.vector, 1, RS).then_inc(s_dve, 1)      # rows 1-17
            cb(nc.vector, 1, RS).then_inc(s_dve, 1)
            # oi 0..8 from a rows 0..17 (row 0 pre-zeroed by GPS)
            nc.vector.tensor_tensor(
                a23a[:], a3[:, 0:RS:2, :], a3[:, 1:RS:2, :], add,
            ).then_inc(s_dve, 1)._wait_ge(s_dve, 2)       # s_dve = 3
            vector.wait_ge(s_b1, 16)
            ctt(nc.vector, RS, HP - 1).then_inc(s_dve, 1)  # rows 18-32
            cb(nc.vector, RS, HP - 1).then_inc(s_dve, 1)
            # oi 9..16 from a rows 18..33 (row 33 pre-zeroed)
            nc.vector.tensor_tensor(
                a23b[:], a3[:, RS:HP:2, :], a3[:, RS + 1:HP:2, :], add,
            ).then_inc(s_dve, 1)._wait_ge(s_dve, 5)       # s_dve = 6
            # stage output half 1 (overlaps matmul 2), then half 2
            vector.wait_ge(s_pe, 1)
            nc.vector.tensor_copy(ot[:, 0:N1], pt0[:, 0:N1]).then_inc(
                s_cp1, 1
            )
            vector.wait_ge(s_pe, 2)
            nc.vector.tensor_copy(
                ot[:, OFF2:OFF2 + N2], pt1[:]
            ).then_inc(s_cp2, 1)

        @block.tensor
        def _(tensor):
            # single-pass fp32r matmuls: E is exactly 0.25, so products are
            # exact; accumulation stays fp32 in PSUM
            tensor.wait_ge(s_dve, 5)
            nc.tensor.matmul(
                pt0[:], etr[:], a2t[:, 0:N1 + 1],
                start=True, stop=True,
            ).then_inc(s_pe, 1)
            tensor.wait_ge(s_dve, 8)
            nc.tensor.matmul(
                pt1[:], etr[:], a2t[:, N1 + 1:N1 + 1 + N2],
                start=True, stop=True,
            ).then_inc(s_pe, 1)

    nc.compile()
    return nc


def _get_program():
    global _PROGRAM
    if _PROGRAM is None:
        _PROGRAM = _build_program()
    return _PROGRAM


def _run(enc_x: np.ndarray, mask: np.ndarray = None, **spmd_kwargs):
    from concourse.bass_utils import run_bass_kernel_spmd

    nc = _get_program()
    in_maps = []
    for i in range(N_CORES):
        sl = slice(i * B_SH, (i + 1) * B_SH)
        in_maps.append({"x": np.ascontiguousarray(enc_x[sl], dtype=np.float32)})
    res = run_bass_kernel_spmd(nc, in_maps, list(range(N_CORES)), **spmd_kwargs)
    out = np.concatenate([res.results[i]["out"] for i in range(N_CORES)], axis=0)
    return out, res


def kernel(enc_x, weight=None, mask=None, **_unused):
    enc_x = np.asarray(enc_x, dtype=np.float32)
    assert enc_x.shape == (B, IN_DIM), enc_x.shape
    out, _ = _run(enc_x)
    return out


# revision 25
# speedup vs baseline: 1.1288x; 1.1288x over previous
"""AvgPool2d-as-Toeplitz kernel for Trainium2 (8 NeuronCores, SPMD).

Reference computes out = (enc_x * mask) @ W.T where W is the dense
Toeplitz matrix of conv2d with kernel ones(C,C,KH,KW)/(KH*KW) over the
flattened zero-padded input (C=16, KH=KW=2, stride 2, pad 1, H=W=32),
and mask zeroes the 1-pixel padding ring of each 34x34 channel image.

Structure exploited:
  W[(co,oi,oj), (ci,i,j)] = 0.25  iff  i in {2oi, 2oi+1} and j in {2oj, 2oj+1}
— independent of co, summed over every ci. With x viewed as
[B, C, 34, 34] and the mask folded in structurally (pooling windows
never read the masked border rows/columns):

  out[b, co, oi, oj] = 0.25 * sum_ci sum_window x[b, ci, i, j]
       over i in {2oi, 2oi+1} ∩ [1,32],  j in {2oj, 2oj+1} ∩ [1,32]

The result is numerically exact (fp32 throughout, ~1e-7 fro error).

Per-core plan (4 batches per core, batch-parallel across 8 cores), raw
bacc with manual semaphores, flat single-block emission (no Block
wrapper: its exit drain+barrier would only delay the NEFF epilogue),
optimized for latency:
  - Input DMA trimmed to image rows 1..32, one DMA per HWDGE queue
    (per-DMA pipe latency of ~1.5us dominates, so fewer/bigger
    transfers win over chunking): SP ring rows 1-17 (output half 1),
    ACT ring rows 18-32.
  - DVE does all vector work (GPSIMD tensor ops need a library swap
    and run ~2x slower; ACT-engine copies trigger an activation-table
    load on the ACT DMA ring): column-pair adds + border-column copies
    + row-pair adds per half, and both PSUM->SBUF copies.  GPSIMD only
    builds E[(b,ci),(b2,co)] = 0.25*(b==b2) with std-lib
    memset/affine_select, off the critical path.
  - PE: two fp32 matmuls (ci-sum + co-broadcast + *0.25), one per
    output half, into separate PSUM banks; half 1 (oi 0..8) launches
    while half 2's input is still in flight.
  - Output DMA split across both queues, issued as soon as each half
    is staged, with NO completion waits: the fixed multi-us NEFF
    teardown (semaphore-reset storm) overlaps the output DMA flight,
    and the runtime drains DGE queues before completion.
  - Engines execute with relaxed ordering, so readers that follow
    same-engine writers carry attached semaphore waits (dropping them
    corrupts the first elements written by the preceding op on cold
    runs).
"""

import sys

import numpy as np

if "/opt/trn_rl_repo" not in sys.path:
    sys.path.insert(0, "/opt/trn_rl_repo")

B, C = 32, 16
HP = WP = 34
OH = OW = 17
IMG = HP * WP             # 1156
IN_DIM = C * IMG          # 18496
OUT_DIM = C * OH * OW     # 4624
N_CORES = 8
B_SH = B // N_CORES       # 4 batches per core
P = B_SH * C              # 64 partitions in use

OI1 = 9                   # output rows in half 1 (oi 0..8 <- image rows 0..17)
N1 = OI1 * OW             # 153
N2 = (OH - OI1) * OW      # 136
GAP = 16                  # sacrificial staging gap between output halves
OFF2 = N1 + GAP           # staging offset of output half 2

RS = 18                   # image-row split: SP ring rows 1..17, ACT rows 18..32

_PROGRAM = None


def _build_program():
    import concourse.bacc as bacc
    import concourse.mybir as mybir

    f32 = mybir.dt.float32
    add = mybir.AluOpType.add
    nc = bacc.Bacc()

    x = nc.declare_dram_parameter("x", [B_SH, IN_DIM], f32, isOutput=False)
    out = nc.declare_dram_parameter("out", [B_SH, OUT_DIM], f32, isOutput=True)
    xv = x[:, :].rearrange("b (c f) -> (b c) f", c=C)   # [64, 1156]
    ov = out[:, :].rearrange("b (co s) -> (b co) s", co=C)

    with (
        nc.sbuf_tensor([P, IMG], f32) as xt,
        nc.sbuf_tensor([P, P], f32) as et,
        nc.sbuf_tensor([P, HP * OW], f32) as at,
        nc.sbuf_tensor([P, N1 + N2], f32) as a2t,
        nc.sbuf_tensor([P, OFF2 + N2], f32) as ot,
        nc.psum_tensor([P, N1], f32) as pt0,
        nc.psum_tensor([P, N2], f32) as pt1,
        nc.semaphore("s_a1") as s_a1,
        nc.semaphore("s_b1") as s_b1,
        nc.semaphore("s_gps") as s_gps,
        nc.semaphore("s_dve") as s_dve,
        nc.semaphore("s_pe") as s_pe,
        nc.semaphore("s_cp1") as s_cp1,
        nc.semaphore("s_cp2") as s_cp2,
        nc.semaphore("s_out") as s_out,
    ):
        x3 = xt[:].rearrange("p (i j) -> p i j", i=HP)
        a3 = at[:].rearrange("p (i oj) -> p i oj", i=HP)
        a23a = a2t[:, 0:N1].rearrange("p (oi oj) -> p oi oj", oi=OI1)
        a23b = a2t[:, N1:N1 + N2].rearrange(
            "p (oi oj) -> p oi oj", oi=OH - OI1
        )
        e3 = et[:].rearrange("p (qb qc) -> p qb qc", qb=B_SH)

        def ctt(eng, r0, r1):
            # column-pair add for interior output columns oj 1..15
            return eng.tensor_tensor(
                a3[:, r0:r1, 1:16],
                x3[:, r0:r1, 2:32:2], x3[:, r0:r1, 3:33:2], add,
            )

        def cb(eng, r0, r1):
            # border output columns oj 0 / 16 <- image columns 1 / 32
            return eng.tensor_copy(
                a3[:, r0:r1, 0:17:16], x3[:, r0:r1, 1:33:31]
            )

        sync = nc.sync
        # rows 1..17 (output half 1) on the SP ring
        sync.dma_start(
            xt[:, WP:RS * WP], xv[:, WP:RS * WP]
        ).then_inc(s_a1, 16)
        # ship output half 2 once staged; no completion wait: the NEFF
        # teardown overlaps the output flight and the runtime drains
        # the DGE queues before completion
        sync.wait_ge(s_cp2, 1)
        sync.dma_start(
            ov[:, N1:OH * OW], ot[:, OFF2:OFF2 + N2], single_packet=True
        ).then_inc(s_out, 16)

        scalar = nc.scalar
        # rows 18..32 (output half 2) on the ACT ring
        scalar.dma_start(
            xt[:, RS * WP:IMG - WP], xv[:, RS * WP:IMG - WP]
        ).then_inc(s_b1, 16)
        scalar.wait_ge(s_cp1, 1)
        scalar.dma_start(
            ov[:, 0:N1], ot[:, 0:N1], single_packet=True
        ).then_inc(s_out, 16)

        gpsimd = nc.gpsimd
        # masked image rows 0 and 33 of the column-pooled tile -> 0,
        # so the row-pair stage needs no border special-casing
        gpsimd.memset(a3[:, 0:HP:HP - 1, :], 0.0).then_inc(s_gps, 1)
        # E[p,(qb,qc)] = 0.25 iff 0 <= p - 16*qb <= 15
        gpsimd.memset(et[:], 0.25).then_inc(s_gps, 1)
        gpsimd.wait_ge(s_gps, 2)
        nc.gpsimd.affine_select(
            e3, e3, [[-C, B_SH], [0, C]], mybir.AluOpType.is_ge, 0.0,
            base=0, channel_multiplier=1,
        ).then_inc(s_gps, 1)
        gpsimd.wait_ge(s_gps, 3)
        nc.gpsimd.affine_select(
            e3, e3, [[C, B_SH], [0, C]], mybir.AluOpType.is_ge, 0.0,
            base=C - 1, channel_multiplier=-1,
        ).then_inc(s_gps, 1)

        vector = nc.vector
        # a3 rows 0/33 must be zeroed before the row-pair adds read them
        vector.wait_ge(s_gps, 1)
        vector.wait_ge(s_a1, 16)
        ctt(nc.vector, 1, RS).then_inc(s_dve, 1)      # rows 1-17
        cb(nc.vector, 1, RS).then_inc(s_dve, 1)
        # oi 0..8 from a rows 0..17 (row 0 pre-zeroed by GPS)
        nc.vector.tensor_tensor(
            a23a[:], a3[:, 0:RS:2, :], a3[:, 1:RS:2, :], add,
        ).then_inc(s_dve, 1)._wait_ge(s_dve, 2)       # s_dve = 3
        vector.wait_ge(s_b1, 16)
        ctt(nc.vector, RS, HP - 1).then_inc(s_dve, 1)  # rows 18-32
        cb(nc.vector, RS, HP - 1).then_inc(s_dve, 1)
        # oi 9..16 from a rows 18..33 (row 33 pre-zeroed)
        nc.vector.tensor_tensor(
            a23b[:], a3[:, RS:HP:2, :], a3[:, RS + 1:HP:2, :], add,
        ).then_inc(s_dve, 1)._wait_ge(s_dve, 5)       # s_dve = 6
        # stage output half 1 (overlaps matmul 2), then half 2
        vector.wait_ge(s_pe, 1)
        nc.vector.tensor_copy(ot[:, 0:N1], pt0[:]).then_inc(
            s_cp1, 1
        )
        vector.wait_ge(s_pe, 2)
        nc.vector.tensor_copy(
            ot[:, OFF2:OFF2 + N2], pt1[:]
        ).then_inc(s_cp2, 1)

        tensor = nc.tensor
        # exact fp32 (2-pass) matmuls, one per output half
        tensor.wait_ge(s_gps, 4)
        tensor.wait_ge(s_dve, 3)
        nc.tensor.matmul(
            pt0[:], et[:], a2t[:, 0:N1],
            start=True, stop=True,
        ).then_inc(s_pe, 1)
        tensor.wait_ge(s_dve, 6)
        nc.tensor.matmul(
            pt1[:], et[:], a2t[:, N1:N1 + N2],
            start=True, stop=True,
        ).then_inc(s_pe, 1)

    nc.compile()
    return nc


def _get_program():
    global _PROGRAM
    if _PROGRAM is None:
        _PROGRAM = _build_program()
    return _PROGRAM


def _run(enc_x: np.ndarray, mask: np.ndarray = None, **spmd_kwargs):
    from concourse.bass_utils import run_bass_kernel_spmd

    nc = _get_program()
    in_maps = []
    for i in range(N_CORES):
        sl = slice(i * B_SH, (i + 1) * B_SH)
        in_maps.append({"x": np.ascontiguousarray(enc_x[sl], dtype=np.float32)})
    res = run_bass_kernel_spmd(nc, in_maps, list(range(N_CORES)), **spmd_kwargs)
    out = np.concatenate([res.results[i]["out"] for i in range(N_CORES)], axis=0)
    return out, res


def kernel(enc_x, weight=None, mask=None, **_unused):
    enc_x = np.asarray(enc_x, dtype=np.float32)
    assert enc_x.shape == (B, IN_DIM), enc_x.shape
    out, _ = _run(enc_x)
    return out



# revision 29
# speedup vs baseline: 1.1459x; 1.0152x over previous
"""AvgPool2d-as-Toeplitz kernel for Trainium2 (8 NeuronCores, SPMD).

Reference computes out = (enc_x * mask) @ W.T where W is the dense
Toeplitz matrix of conv2d with kernel ones(C,C,KH,KW)/(KH*KW) over the
flattened zero-padded input (C=16, KH=KW=2, stride 2, pad 1, H=W=32),
and mask zeroes the 1-pixel padding ring of each 34x34 channel image.

Structure exploited:
  W[(co,oi,oj), (ci,i,j)] = 0.25  iff  i in {2oi, 2oi+1} and j in {2oj, 2oj+1}
— independent of co, summed over every ci. With x viewed as
[B, C, 34, 34] and the mask folded in structurally (pooling windows
never read the masked border rows/columns):

  out[b, co, oi, oj] = 0.25 * sum_ci sum_window x[b, ci, i, j]
       over i in {2oi, 2oi+1} ∩ [1,32],  j in {2oj, 2oj+1} ∩ [1,32]

The result is numerically exact (fp32 throughout, ~1e-7 fro error).

Per-core plan (4 batches per core, batch-parallel across 8 cores), raw
bacc with manual semaphores, flat single-block emission (no Block
wrapper: its exit drain+barrier would only delay the NEFF epilogue),
optimized for latency:
  - Input DMA trimmed to image rows 1..32, one DMA per HWDGE queue
    (per-DMA pipe latency of ~1.5us dominates, so fewer/bigger
    transfers win over chunking): SP ring rows 1-19 (output half 1,
    oi 0..9), ACT ring rows 20-32 (half 2, oi 10..16); the split leaves
    half 2's tail (col/row adds, matmul, copy, DMA issue) slightly
    smaller since it sits on the critical path.
  - DVE does all vector work (GPSIMD tensor ops need a library swap
    and run ~2x slower; ACT-engine copies trigger an activation-table
    load on the ACT DMA ring): column-pair adds + border-column copies
    + row-pair adds per half, and both PSUM->SBUF copies.  GPSIMD only
    builds E[(b,ci),(b2,co)] = 0.25*(b==b2) with std-lib
    memset/affine_select, off the critical path.
  - PE: two fp32 matmuls (ci-sum + co-broadcast + *0.25), one per
    output half, into separate PSUM banks; half 1 launches while
    half 2's input is still in flight.
  - Output DMA split across both queues, issued as soon as each half
    is staged, with NO completion waits: the fixed multi-us NEFF
    teardown (semaphore-reset storm) overlaps the output DMA flight,
    and the runtime drains DGE queues before completion.
  - Engines execute with relaxed ordering, so readers that follow
    same-engine writers carry attached semaphore waits (dropping them
    corrupts the first elements written by the preceding op on cold
    runs).  Cross-engine waits are attached to their consuming
    instructions rather than standalone, saving a dispatch slot at
    each critical hand-off.
"""

import sys

import numpy as np

if "/opt/trn_rl_repo" not in sys.path:
    sys.path.insert(0, "/opt/trn_rl_repo")

B, C = 32, 16
HP = WP = 34
OH = OW = 17
IMG = HP * WP             # 1156
IN_DIM = C * IMG          # 18496
OUT_DIM = C * OH * OW     # 4624
N_CORES = 8
B_SH = B // N_CORES       # 4 batches per core
P = B_SH * C              # 64 partitions in use

OI1 = 10                  # output rows in half 1 (oi 0..9 <- image rows 0..19)
N1 = OI1 * OW             # 170
N2 = (OH - OI1) * OW      # 119
GAP = 16                  # sacrificial staging gap between output halves
OFF2 = N1 + GAP           # staging offset of output half 2

RS = 20                   # image-row split: SP ring rows 1..19, ACT rows 20..32

_PROGRAM = None


def _build_program():
    import concourse.bacc as bacc
    import concourse.mybir as mybir

    f32 = mybir.dt.float32
    add = mybir.AluOpType.add
    nc = bacc.Bacc()

    x = nc.declare_dram_parameter("x", [B_SH, IN_DIM], f32, isOutput=False)
    out = nc.declare_dram_parameter("out", [B_SH, OUT_DIM], f32, isOutput=True)
    xv = x[:, :].rearrange("b (c f) -> (b c) f", c=C)   # [64, 1156]
    ov = out[:, :].rearrange("b (co s) -> (b co) s", co=C)

    with (
        nc.sbuf_tensor([P, IMG], f32) as xt,
        nc.sbuf_tensor([P, P], f32) as et,
        nc.sbuf_tensor([P, HP * OW], f32) as at,
        nc.sbuf_tensor([P, N1 + N2], f32) as a2t,
        nc.sbuf_tensor([P, OFF2 + N2], f32) as ot,
        nc.psum_tensor([P, N1], f32) as pt0,
        nc.psum_tensor([P, N2], f32) as pt1,
        nc.semaphore("s_a1") as s_a1,
        nc.semaphore("s_b1") as s_b1,
        nc.semaphore("s_gps") as s_gps,
        nc.semaphore("s_dve") as s_dve,
        nc.semaphore("s_pe") as s_pe,
        nc.semaphore("s_cp1") as s_cp1,
        nc.semaphore("s_cp2") as s_cp2,
        nc.semaphore("s_out") as s_out,
    ):
        x3 = xt[:].rearrange("p (i j) -> p i j", i=HP)
        a3 = at[:].rearrange("p (i oj) -> p i oj", i=HP)
        a23a = a2t[:, 0:N1].rearrange("p (oi oj) -> p oi oj", oi=OI1)
        a23b = a2t[:, N1:N1 + N2].rearrange(
            "p (oi oj) -> p oi oj", oi=OH - OI1
        )
        e3 = et[:].rearrange("p (qb qc) -> p qb qc", qb=B_SH)

        def ctt(eng, r0, r1):
            # column-pair add for interior output columns oj 1..15
            return eng.tensor_tensor(
                a3[:, r0:r1, 1:16],
                x3[:, r0:r1, 2:32:2], x3[:, r0:r1, 3:33:2], add,
            )

        def cb(eng, r0, r1):
            # border output columns oj 0 / 16 <- image columns 1 / 32
            return eng.tensor_copy(
                a3[:, r0:r1, 0:17:16], x3[:, r0:r1, 1:33:31]
            )

        sync = nc.sync
        # rows 1..17 (output half 1) on the SP ring
        sync.dma_start(
            xt[:, WP:RS * WP], xv[:, WP:RS * WP]
        ).then_inc(s_a1, 16)
        # ship output half 2 once staged; no completion wait: the NEFF
        # teardown overlaps the output flight and the runtime drains
        # the DGE queues before completion
        sync.dma_start(
            ov[:, N1:OH * OW], ot[:, OFF2:OFF2 + N2], single_packet=True
        ).then_inc(s_out, 16)._wait_ge(s_cp2, 1)

        scalar = nc.scalar
        # rows 18..32 (output half 2) on the ACT ring
        scalar.dma_start(
            xt[:, RS * WP:IMG - WP], xv[:, RS * WP:IMG - WP]
        ).then_inc(s_b1, 16)
        scalar.dma_start(
            ov[:, 0:N1], ot[:, 0:N1], single_packet=True
        ).then_inc(s_out, 16)._wait_ge(s_cp1, 1)

        gpsimd = nc.gpsimd
        # masked image rows 0 and 33 of the column-pooled tile -> 0,
        # so the row-pair stage needs no border special-casing
        gpsimd.memset(a3[:, 0:HP:HP - 1, :], 0.0).then_inc(s_gps, 1)
        # E[p,(qb,qc)] = 0.25 iff 0 <= p - 16*qb <= 15
        gpsimd.memset(et[:], 0.25).then_inc(s_gps, 1)
        gpsimd.wait_ge(s_gps, 2)
        nc.gpsimd.affine_select(
            e3, e3, [[-C, B_SH], [0, C]], mybir.AluOpType.is_ge, 0.0,
            base=0, channel_multiplier=1,
        ).then_inc(s_gps, 1)
        gpsimd.wait_ge(s_gps, 3)
        nc.gpsimd.affine_select(
            e3, e3, [[C, B_SH], [0, C]], mybir.AluOpType.is_ge, 0.0,
            base=C - 1, channel_multiplier=-1,
        ).then_inc(s_gps, 1)

        vector = nc.vector
        # a3 rows 0/33 must be zeroed before the row-pair adds read them
        vector.wait_ge(s_gps, 1)
        ctt(nc.vector, 1, RS).then_inc(s_dve, 1)._wait_ge(
            s_a1, 16
        )                                             # rows 1..RS-1
        cb(nc.vector, 1, RS).then_inc(s_dve, 1)
        # oi 0..8 from a rows 0..17 (row 0 pre-zeroed by GPS)
        nc.vector.tensor_tensor(
            a23a[:], a3[:, 0:RS:2, :], a3[:, 1:RS:2, :], add,
        ).then_inc(s_dve, 1)._wait_ge(s_dve, 2)       # s_dve = 3
        ctt(nc.vector, RS, HP - 1).then_inc(s_dve, 1)._wait_ge(
            s_b1, 16
        )                                              # rows RS..32
        cb(nc.vector, RS, HP - 1).then_inc(s_dve, 1)
        # oi 9..16 from a rows 18..33 (row 33 pre-zeroed)
        nc.vector.tensor_tensor(
            a23b[:], a3[:, RS:HP:2, :], a3[:, RS + 1:HP:2, :], add,
        ).then_inc(s_dve, 1)._wait_ge(s_dve, 5)       # s_dve = 6
        # stage output half 1 (overlaps matmul 2), then half 2
        nc.vector.tensor_copy(ot[:, 0:N1], pt0[:]).then_inc(
            s_cp1, 1
        )._wait_ge(s_pe, 1)
        nc.vector.tensor_copy(
            ot[:, OFF2:OFF2 + N2], pt1[:]
        ).then_inc(s_cp2, 1)._wait_ge(s_pe, 2)

        tensor = nc.tensor
        # exact fp32 (2-pass) matmuls, one per output half
        tensor.wait_ge(s_gps, 4)
        tensor.wait_ge(s_dve, 3)
        nc.tensor.matmul(
            pt0[:], et[:], a2t[:, 0:N1],
            start=True, stop=True,
        ).then_inc(s_pe, 1)
        tensor.wait_ge(s_dve, 6)
        nc.tensor.matmul(
            pt1[:], et[:], a2t[:, N1:N1 + N2],
            start=True, stop=True,
        ).then_inc(s_pe, 1)

    nc.compile()
    return nc


def _get_program():
    global _PROGRAM
    if _PROGRAM is None:
        _PROGRAM = _build_program()
    return _PROGRAM


def _run(enc_x: np.ndarray, mask: np.ndarray = None, **spmd_kwargs):
    from concourse.bass_utils import run_bass_kernel_spmd

    nc = _get_program()
    in_maps = []
    for i in range(N_CORES):
        sl = slice(i * B_SH, (i + 1) * B_SH)
        in_maps.append({"x": np.ascontiguousarray(enc_x[sl], dtype=np.float32)})
    res = run_bass_kernel_spmd(nc, in_maps, list(range(N_CORES)), **spmd_kwargs)
    out = np.concatenate([res.results[i]["out"] for i in range(N_CORES)], axis=0)
    return out, res


def kernel(enc_x, weight=None, mask=None, **_unused):
    enc_x = np.asarray(enc_x, dtype=np.float32)
    assert enc_x.shape == (B, IN_DIM), enc_x.shape
    out, _ = _run(enc_x)
    return out

